# revision 20
# baseline (speedup 1.0000x reference)
"""AtomAttention Trainium2 kernel (fp8 DoubleRow, v4).

reference:
    bias = adj + dist + coulomb                      # [B, N, N]
    q = m @ Wq.T + bq; k = m @ Wk.T + bk; v = m @ Wv.T + bv
    attn = softmax(q @ k.T / sqrt(H) + bias, axis=-1)
    out  = attn @ v + m                              # [B, N, H]

B=16, N=1024, H=128.  Data-parallel over batch: 2 batches per core on 8
NeuronCores.  Bias tensors dominate HBM traffic -> shipped pre-scaled by
sqrt(H) in fp8e4m3 (6 MB/core vs 24 MB f32); the 1/sqrt(H) is re-applied
inside the exp; a global shift C keeps E in fp8 range and cancels in
softmax.

Engine plan:
  - fp8 DoubleRow matmuls (two contractions per pass):
      DR1: (kT_j | I) x (qT | b0_j)   -> k.T q + b0
      DR2: (I | I)    x (b1_j | b2_j) -> + b1 + b2
    PV DoubleRows pair consecutive j-blocks; softmax denominator comes
    from a ones column in v.
  - scores live in single-bank [128,512] PSUM tiles (bufs=5) so several
    j-halves are in flight; exp of each half alternates between ACT
    (activation Exp) and DVE (Schraudolph bit trick: clamp(round(A*psum
    + B)) written as uint8 == fp8e4m3 bits of exp).  Total rel err
    ~5.4e-3 (gate 2e-2).
  - PV for chunk c issues after scores of chunk c+1 so the in-order PE
    never waits on exp; PV accumulates across chunks in 3 PSUM banks.
  - mega 25-slot bias tile per batch (slot 0 = qT) -> chunk DMAs never
    have WAR hazards; descriptor generation is spread across the three
    DMA rings (sync=bias, scalar=weights+mT, gpsimd=mn/identity/out).
"""

import sys
import types

import numpy as np

B, N, H = 16, 1024, 128
NB = N // 128   # 8 row blocks
NCH = 4         # chunks per batch, 2 blocks each
BPC = 2         # batches per core
NCORES = 8
SCALE = float(1.0 / np.sqrt(np.float32(H)))   # 0.08838835
INV = float(np.sqrt(np.float32(H)))           # 11.313708
CSHIFT = 3.25                                 # global exp shift
LOG2E = 1.4426950408889634
EXP_A = SCALE * LOG2E * 8.0                   # 1.0201033
EXP_B = 8.0 * (7.0 - CSHIFT * LOG2E) - 0.4    # Schraudolph, RNE convert

_CACHE = {}


def _install_ntff_hook():
    """The agent image's antenv lacks axon_hooks; register the NTFF
    profiling hook manually so trace=True yields exec_time_ns."""
    if "antenv.axon_hooks" in sys.modules:
        return
    try:
        import trn_agent_boot.trn_boot as tb

        hook = tb._ntff_profile_via_ctypes("/opt/axon/libaxon_pjrt.so")
    except Exception:
        hook = None
    mod = types.ModuleType("antenv.axon_hooks")
    mod.get_axon_ntff_profile_hook = lambda: hook
    mod.set_axon_ntff_profile_hook = lambda h: None
    sys.modules["antenv.axon_hooks"] = mod


def _build():
    if "nc" in _CACHE:
        return _CACHE["nc"]
    import concourse.bass as bass
    from concourse import bacc, mybir, tile

    f32 = mybir.dt.float32
    bf16 = mybir.dt.bfloat16
    fp8 = mybir.dt.float8e4
    u8 = mybir.dt.uint8
    ts = bass.ts
    DR = mybir.MatmulPerfMode.DoubleRow
    Exp = mybir.ActivationFunctionType.Exp
    Alu = mybir.AluOpType

    nc = bacc.Bacc("TRN2", target_bir_lowering=False, debug=False)

    # mT with f32 bk|bq appended as four bf16 columns (rides the same DMA)
    mT_d = nc.dram_tensor("mT", [BPC, 128, N + 4], bf16, kind="ExternalInput")
    mn_d = nc.dram_tensor("mn", [BPC, N, H], bf16, kind="ExternalInput")
    # [b, c, p, 6 slots (s*3+t), n] fp8, partition-major for 6KB/row DMA
    bias_d = nc.dram_tensor("biasq", [BPC, NCH, 128, 6, N], fp8,
                            kind="ExternalInput")
    ii8_d = nc.dram_tensor("ii8", [128, N], fp8, kind="ExternalInput")
    wq_d = nc.dram_tensor("wq", [H, H], bf16, kind="ExternalInput")
    wk_d = nc.dram_tensor("wk", [H, H], bf16, kind="ExternalInput")
    wv_d = nc.dram_tensor("wv", [H, H], bf16, kind="ExternalInput")
    bv_d = nc.dram_tensor("bv", [1, H], bf16, kind="ExternalInput")
    out_d = nc.dram_tensor("out", [BPC, N, H], bf16, kind="ExternalOutput")

    mn_r = mn_d.rearrange("b (i p) h -> b p i h", p=128)
    out_r = out_d.rearrange("b (i p) h -> b p i h", p=128)

    with tile.TileContext(nc) as tc:
        with (
            tc.tile_pool(name="const", bufs=1) as const,
            tc.tile_pool(name="cb", bufs=1) as cbp,
            tc.tile_pool(name="sb", bufs=2) as sb,
            tc.tile_pool(name="epool", bufs=3) as epool,
            tc.tile_pool(name="work", bufs=6) as work,
            tc.tile_pool(name="pqk", bufs=5, space="PSUM") as pqk,
            tc.tile_pool(name="po", bufs=3, space="PSUM") as pop,
        ):
            # ---- allocate big per-batch tiles ----
            btiles = [cbp.tile([128, 25, N], fp8, name=f"bt{b}")
                      for b in range(BPC)]
            kids = [cbp.tile([128, 2, N], fp8, name=f"kid{b}")
                    for b in range(BPC)]
            vaugs = [cbp.tile([128, NB, 132], fp8, name=f"va{b}")
                     for b in range(BPC)]
            mT_ts = [sb.tile([128, N + 4], bf16, name=f"mT{b}", tag="mT")
                     for b in range(BPC)]
            mn_ts = [sb.tile([128, NB, H], bf16, name=f"mn{b}", tag="mn")
                     for b in range(BPC)]
            mb_ts = [sb.tile([128, NB, H], bf16, name=f"mb{b}", tag="mb")
                     for b in range(BPC)]

            # ---- descriptor generation spread across the 3 DMA rings ----
            # scalar ring: bf16 weights (tiny, land first)
            wk_b = const.tile([128, 128], bf16)
            wq_b = const.tile([128, 128], bf16)
            wv_b = const.tile([128, 128], bf16)
            nc.scalar.dma_start(out=wk_b, in_=wk_d[:, :])
            nc.scalar.dma_start(out=wq_b, in_=wq_d[:, :])
            nc.scalar.dma_start(out=wv_b, in_=wv_d[:, :])
            # sync ring: identity planes + mT ahead of the bias flood
            # (queue FIFO), then bias in half-chunks for deep pipelining
            nc.sync.dma_start(out=kids[0][:, 1, :], in_=ii8_d[:, :])
            nc.sync.dma_start(out=mT_ts[0], in_=mT_d[0])
            nc.sync.dma_start(out=kids[1][:, 1, :], in_=ii8_d[:, :])
            nc.sync.dma_start(out=mT_ts[1], in_=mT_d[1])
            for b in range(BPC):
                for c in range(NCH):
                    for s in range(2):
                        sl = 1 + 6 * c + 3 * s
                        nc.sync.dma_start(
                            out=btiles[b][:, sl:sl + 3, :],
                            in_=bias_d[b, c, :, 3 * s:3 * s + 3, :])
            # gpsimd ring: identity planes, mn, bv broadcast
            bvb = const.tile([128, 128], bf16)
            bv_ap = bv_d[:, :]
            bv_bcast = bass.AP(
                tensor=bv_ap.tensor,
                offset=bv_ap.offset,
                ap=[[0, 128]] + list(bv_ap.ap[1:]),
            )
            nc.gpsimd.dma_start(out=mn_ts[0], in_=mn_r[0])
            nc.gpsimd.dma_start(out=mn_ts[1], in_=mn_r[1])
            nc.gpsimd.dma_start(out=bvb, in_=bv_bcast)

            # ---- DVE-side constants ----
            # scratch for PE clock warm-up: dummy matmuls keep the PE busy
            # from preamble end until mT arrives, so the p-state is ramped
            # when real work starts
            scr8 = const.tile([128, 512], fp8)
            nc.vector.memset(scr8, 0.0)
            pscr = pqk.tile([128, 512], f32, name="pscr", tag="pqk")
            for _ in range(8):
                nc.tensor.matmul(pscr, lhsT=scr8[:, 0:128], rhs=scr8,
                                 start=True, stop=True, skip_group_check=True)
            negC = const.tile([128, 1], f32)
            nc.vector.memset(negC, -CSHIFT)
            for b in range(BPC):
                nc.vector.memset(vaugs[b][:, :, 128:129], 1.0)

            # residual prep on gpsimd (idle early)
            for b in range(BPC):
                for i in range(NB):
                    nc.gpsimd.tensor_add(mb_ts[b][:, i], mn_ts[b][:, i], bvb)

            # ---- projection helpers ----
            def proj_kq(b):
                mT_b = mT_ts[b]
                for h in range(2):
                    ps_kh = pqk.tile([128, 512], f32, name=f"ps_k{b}_{h}",
                                     tag="pqk")
                    nc.tensor.matmul(ps_kh, lhsT=wk_b, rhs=mT_b[:, ts(h, 512)],
                                     start=True, stop=True)
                    nc.vector.tensor_scalar_add(
                        kids[b][:, 0, ts(h, 512)], ps_kh,
                        mT_ts[b][:, N:N + 2].bitcast(f32))
                for h in range(2):
                    ps_qh = pqk.tile([128, 512], f32, name=f"ps_q{b}_{h}",
                                     tag="pqk")
                    nc.tensor.matmul(ps_qh, lhsT=wq_b, rhs=mT_b[:, ts(h, 512)],
                                     start=True, stop=True)
                    nc.vector.tensor_scalar_add(
                        btiles[b][:, 0, ts(h, 512)], ps_qh,
                        mT_ts[b][:, N + 2:N + 4].bitcast(f32))

            def proj_v(b):
                mT_b = mT_ts[b]
                for h in range(2):
                    ps_vh = pqk.tile([128, 512], f32, name=f"ps_v{b}_{h}",
                                     tag="pqk")
                    for i in range(4):
                        nc.tensor.matmul(ps_vh[:, ts(i, 128)],
                                         lhsT=mT_b[:, ts(4 * h + i, 128)],
                                         rhs=wv_b,
                                         start=(i == 0), stop=(i == 3),
                                         skip_group_check=True)
                    for i in range(4):
                        nc.vector.tensor_copy(vaugs[b][:, 4 * h + i, 0:128],
                                              ps_vh[:, ts(i, 128)])

            # ---- per-batch chunk machinery ----
            state = {}

            def setup_batch(b):
                bt = btiles[b]
                kid = kids[b]
                ps_os = [
                    pop.tile([128, 3, 132], f32, name=f"ps_o{b}_{p}", tag="po")
                    for p in range(3)
                ]
                base = bt[:, 0, 0]
                pstride = list(base.ap[0])
                ibase = kid[:, 1, 0]
                ii_w = bass.AP(tensor=ibase.tensor, offset=ibase.offset,
                               ap=[list(ibase.ap[0]), [128, 2], [1, 128]])
                state[b] = (bt, kid, ps_os, base, pstride, ii_w, [])

            def scores(b, c):
                bt, kid, ps_os, base, pstride, ii_w, Es = state[b]
                E = epool.tile([128, 2, N], fp8, name=f"E{b}_{c}", tag="E")
                for s in range(2):
                    j = 2 * c + s
                    slot = 1 + 6 * c + 3 * s
                    pss = [pqk.tile([128, 512], f32,
                                    name=f"ps_s{b}_{j}_{h}", tag="pqk")
                           for h in range(2)]
                    for h in range(2):
                        rhs1 = bass.AP(
                            tensor=base.tensor,
                            offset=base.offset + 512 * h,
                            ap=[pstride, [slot * N, 2], [1, 512]])
                        nc.tensor.matmul(pss[h],
                                         lhsT=kid[:, :, ts(j, 128)],
                                         rhs=rhs1, start=True, stop=False,
                                         perf_mode=DR,
                                         skip_group_check=True)
                    for h in range(2):
                        rhs2 = bass.AP(
                            tensor=base.tensor,
                            offset=base.offset + (slot + 1) * N + 512 * h,
                            ap=[pstride, [N, 2], [1, 512]])
                        nc.tensor.matmul(pss[h], lhsT=ii_w, rhs=rhs2,
                                         start=False, stop=True,
                                         perf_mode=DR,
                                         skip_group_check=True)
                    for h in range(2):
                        eh = E[:, s, ts(h, 512)]
                        if h == 0:
                            nc.scalar.activation(out=eh, in_=pss[h],
                                                 func=Exp, bias=negC,
                                                 scale=SCALE)
                        else:
                            nc.vector.tensor_scalar(
                                out=eh.bitcast(u8), in0=pss[h],
                                scalar1=EXP_A, scalar2=EXP_B,
                                op0=Alu.mult, op1=Alu.add)
                Es.append(E)

            def pv(b, c):
                bt, kid, ps_os, base, pstride, ii_w, Es = state[b]
                E = Es[c]
                va = vaugs[b]
                for i in range(NB):
                    nc.tensor.matmul(
                        ps_os[i // 3][:, i % 3, 0:129],
                        lhsT=E[:, :, ts(i, 128)],
                        rhs=va[:, 2 * c:2 * c + 2, 0:129],
                        start=(c == 0 and i % 3 == 0),
                        stop=(c == NCH - 1),
                        perf_mode=DR, skip_group_check=True)

            def tail(b):
                bt, kid, ps_os, base, pstride, ii_w, Es = state[b]
                ob = sb.tile([128, NB, H], bf16, name=f"ob{b}", tag="ob")
                mb_t = mb_ts[b]
                # all reciprocals first so the mul/add pipeline never
                # ping-pongs; muls split ACT/DVE, adds split gpsimd/DVE
                rs, o1s = [], []
                for i in range(NB):
                    ps_o = ps_os[i // 3][:, i % 3]
                    r = work.tile([128, 1], f32, name=f"r{b}_{i}", tag="r")
                    nc.vector.reciprocal(r, ps_o[:, 128:129])
                    rs.append(r)
                for i in range(NB):
                    ps_o = ps_os[i // 3][:, i % 3]
                    o1 = work.tile([128, 128], f32, name=f"o1_{b}_{i}",
                                   tag="o1")
                    nc.scalar.mul(o1, ps_o[:, 0:128], rs[i])
                    nc.vector.tensor_add(ob[:, i], o1, mb_t[:, i])
                    if i % 2 == 1:
                        nc.gpsimd.dma_start(out=out_r[b, :, i - 1:i + 1],
                                            in_=ob[:, i - 1:i + 1])

            # ---- schedule (v5 order): all projections up front; b1's
            # PV accumulators reuse b0's PSUM banks, so b1 is set up only
            # after b0's tail has read them
            proj_kq(0)
            proj_v(0)
            proj_kq(1)
            proj_v(1)
            setup_batch(0)
            scores(0, 0)
            for c in range(1, NCH):
                scores(0, c)
                pv(0, c - 1)
            pv(0, NCH - 1)
            tail(0)
            setup_batch(1)
            for c in range(NCH):
                scores(1, c)
                if c > 0:
                    pv(1, c - 1)
            pv(1, NCH - 1)
            tail(1)

    nc.compile()
    _CACHE["nc"] = nc
    return nc


def _shard_inputs(m, adj, dist, coulomb, Wq, bq, Wk, bk, Wv, bv):
    import ml_dtypes

    e4 = ml_dtypes.float8_e4m3
    bf = ml_dtypes.bfloat16

    wqT = np.ascontiguousarray(Wq.T).astype(bf)
    wkT = np.ascontiguousarray(Wk.T).astype(bf)
    wvT = np.ascontiguousarray(Wv.T).astype(bf)
    bv_s = bv.astype(bf).reshape(1, H)

    mT = np.ascontiguousarray(np.swapaxes(m, 1, 2)).astype(bf)
    bits = np.stack([bk, bq], axis=1).astype(np.float32).view(bf)  # [H, 4]
    ext = np.broadcast_to(bits[None], (B, H, 4))
    mT = np.ascontiguousarray(np.concatenate([mT, ext], axis=2))
    mn_b = np.ascontiguousarray(m).astype(bf)

    # bias: transpose to key-major, scale by sqrt(H), fp8, interleave to
    # [B, NCH, 128, 6(s*3+t), N] partition-major
    stack = np.stack([np.swapaxes(t, 1, 2) for t in (adj, dist, coulomb)],
                     axis=2)                       # [B, Nk, 3, Nq]
    stack = stack.reshape(B, NCH, 2, 128, 3, N)    # [B, c, s, p, t, n]
    stack = stack.transpose(0, 1, 3, 2, 4, 5)      # [B, c, p, s, t, n]
    biasq = np.ascontiguousarray(stack * INV).astype(e4)
    biasq = biasq.reshape(B, NCH, 128, 6, N)

    I8 = np.eye(128, dtype=np.float32).astype(e4)
    ii8 = np.ascontiguousarray(np.tile(I8, (1, NB)))

    in_maps = []
    for c in range(NCORES):
        sl = slice(c * BPC, (c + 1) * BPC)
        in_maps.append({
            "mT": mT[sl],
            "mn": mn_b[sl],
            "biasq": biasq[sl],
            "wq": wqT, "wk": wkT, "wv": wvT, "bv": bv_s,
            "ii8": ii8,
        })
    return in_maps


def run(trace=False, **inputs):
    _install_ntff_hook()
    from concourse.bass_utils import run_bass_kernel_spmd

    nc = _build()
    in_maps = _shard_inputs(**inputs)
    try:
        res = run_bass_kernel_spmd(nc, in_maps, core_ids=list(range(NCORES)),
                                   trace=trace)
    except Exception:
        # transient device errors (NRT_EXEC_UNIT_UNRECOVERABLE) have been
        # observed on this fabric; one retry usually succeeds
        res = run_bass_kernel_spmd(nc, in_maps, core_ids=list(range(NCORES)),
                                   trace=trace)
    out = np.concatenate([r["out"] for r in res.results], axis=0)
    return out.astype(np.float32), res


def kernel(**inputs):
    inputs = {k: np.asarray(v) for k, v in inputs.items()}
    out, _ = run(trace=False, **inputs)
    return out


# revision 21
# speedup vs baseline: 1.0494x; 1.0494x over previous
"""AtomAttention Trainium2 kernel (fp8 DoubleRow, v4).

reference:
    bias = adj + dist + coulomb                      # [B, N, N]
    q = m @ Wq.T + bq; k = m @ Wk.T + bk; v = m @ Wv.T + bv
    attn = softmax(q @ k.T / sqrt(H) + bias, axis=-1)
    out  = attn @ v + m                              # [B, N, H]

B=16, N=1024, H=128.  Data-parallel over batch: 2 batches per core on 8
NeuronCores.  Bias tensors dominate HBM traffic -> shipped pre-scaled by
sqrt(H) in fp8e4m3 (6 MB/core vs 24 MB f32); the 1/sqrt(H) is re-applied
inside the exp; a global shift C keeps E in fp8 range and cancels in
softmax.

Engine plan:
  - fp8 DoubleRow matmuls (two contractions per pass):
      DR1: (kT_j | I) x (qT | b0_j)   -> k.T q + b0
      DR2: (I | I)    x (b1_j | b2_j) -> + b1 + b2
    PV DoubleRows pair consecutive j-blocks; softmax denominator comes
    from a ones column in v.
  - scores live in single-bank [128,512] PSUM tiles (bufs=5) so several
    j-halves are in flight; exp of each half alternates between ACT
    (activation Exp) and DVE (Schraudolph bit trick: clamp(round(A*psum
    + B)) written as uint8 == fp8e4m3 bits of exp).  Total rel err
    ~5.4e-3 (gate 2e-2).
  - PV for chunk c issues after scores of chunk c+1 so the in-order PE
    never waits on exp; PV accumulates across chunks in 3 PSUM banks.
  - mega 25-slot bias tile per batch (slot 0 = qT) -> chunk DMAs never
    have WAR hazards; descriptor generation is spread across the three
    DMA rings (sync=bias, scalar=weights+mT, gpsimd=mn/identity/out).
"""

import sys
import types

import numpy as np

B, N, H = 16, 1024, 128
NB = N // 128   # 8 row blocks
NCH = 4         # chunks per batch, 2 blocks each
BPC = 2         # batches per core
NCORES = 8
SCALE = float(1.0 / np.sqrt(np.float32(H)))   # 0.08838835
INV = float(np.sqrt(np.float32(H)))           # 11.313708
CSHIFT = 3.25                                 # global exp shift
LOG2E = 1.4426950408889634
EXP_A = SCALE * LOG2E * 8.0                   # 1.0201033
EXP_B = 8.0 * (7.0 - CSHIFT * LOG2E) - 0.4    # Schraudolph, RNE convert

_CACHE = {}


def _install_ntff_hook():
    """The agent image's antenv lacks axon_hooks; register the NTFF
    profiling hook manually so trace=True yields exec_time_ns."""
    if "antenv.axon_hooks" in sys.modules:
        return
    try:
        import trn_agent_boot.trn_boot as tb

        hook = tb._ntff_profile_via_ctypes("/opt/axon/libaxon_pjrt.so")
    except Exception:
        hook = None
    mod = types.ModuleType("antenv.axon_hooks")
    mod.get_axon_ntff_profile_hook = lambda: hook
    mod.set_axon_ntff_profile_hook = lambda h: None
    sys.modules["antenv.axon_hooks"] = mod


def _build():
    if "nc" in _CACHE:
        return _CACHE["nc"]
    import concourse.bass as bass
    from concourse import bacc, mybir, tile

    f32 = mybir.dt.float32
    bf16 = mybir.dt.bfloat16
    fp8 = mybir.dt.float8e4
    u8 = mybir.dt.uint8
    ts = bass.ts
    DR = mybir.MatmulPerfMode.DoubleRow
    Exp = mybir.ActivationFunctionType.Exp
    Alu = mybir.AluOpType

    nc = bacc.Bacc("TRN2", target_bir_lowering=False, debug=False)

    # mT with f32 bk|bq appended as four bf16 columns (rides the same DMA)
    mT_d = nc.dram_tensor("mT", [BPC, 128, N + 4], bf16, kind="ExternalInput")
    mn_d = nc.dram_tensor("mn", [BPC, N, H], bf16, kind="ExternalInput")
    # [b, c, p, 6 slots (s*3+t), n] fp8, partition-major for 6KB/row DMA
    bias_d = nc.dram_tensor("biasq", [BPC, NCH, 128, 6, N], fp8,
                            kind="ExternalInput")
    ii8_d = nc.dram_tensor("ii8", [128, N], fp8, kind="ExternalInput")
    wq_d = nc.dram_tensor("wq", [H, H], bf16, kind="ExternalInput")
    wk_d = nc.dram_tensor("wk", [H, H], bf16, kind="ExternalInput")
    wv_d = nc.dram_tensor("wv", [H, H], bf16, kind="ExternalInput")
    bv_d = nc.dram_tensor("bv", [1, H], bf16, kind="ExternalInput")
    out_d = nc.dram_tensor("out", [BPC, N, H], bf16, kind="ExternalOutput")

    mn_r = mn_d.rearrange("b (i p) h -> b p i h", p=128)
    out_r = out_d.rearrange("b (i p) h -> b p i h", p=128)

    with tile.TileContext(nc) as tc:
        with (
            tc.tile_pool(name="const", bufs=1) as const,
            tc.tile_pool(name="cb", bufs=1) as cbp,
            tc.tile_pool(name="sb", bufs=2) as sb,
            tc.tile_pool(name="epool", bufs=3) as epool,
            tc.tile_pool(name="work", bufs=6) as work,
            tc.tile_pool(name="pqk", bufs=5, space="PSUM") as pqk,
            tc.tile_pool(name="po", bufs=3, space="PSUM") as pop,
        ):
            # ---- allocate big per-batch tiles ----
            btiles = [cbp.tile([128, 25, N], fp8, name=f"bt{b}")
                      for b in range(BPC)]
            kids = [cbp.tile([128, 2, N], fp8, name=f"kid{b}")
                    for b in range(BPC)]
            vaugs = [cbp.tile([128, NB, 132], fp8, name=f"va{b}")
                     for b in range(BPC)]
            mT_ts = [sb.tile([128, N + 4], bf16, name=f"mT{b}", tag="mT")
                     for b in range(BPC)]
            mn_ts = [sb.tile([128, NB, H], bf16, name=f"mn{b}", tag="mn")
                     for b in range(BPC)]
            mb_ts = [sb.tile([128, NB, H], bf16, name=f"mb{b}", tag="mb")
                     for b in range(BPC)]

            # ---- descriptor generation spread across the 3 DMA rings ----
            # scalar ring: bf16 weights (tiny, land first)
            wk_b = const.tile([128, 128], bf16)
            wq_b = const.tile([128, 128], bf16)
            wv_b = const.tile([128, 128], bf16)
            nc.scalar.dma_start(out=wk_b, in_=wk_d[:, :])
            nc.scalar.dma_start(out=wq_b, in_=wq_d[:, :])
            nc.scalar.dma_start(out=wv_b, in_=wv_d[:, :])
            # sync ring: identity planes + mT ahead of the bias flood
            # (queue FIFO), then bias in half-chunks for deep pipelining
            nc.sync.dma_start(out=kids[0][:, 1, :], in_=ii8_d[:, :])
            nc.sync.dma_start(out=mT_ts[0], in_=mT_d[0])
            nc.sync.dma_start(out=kids[1][:, 1, :], in_=ii8_d[:, :])
            nc.sync.dma_start(out=mT_ts[1], in_=mT_d[1])
            for b in range(BPC):
                for c in range(NCH):
                    for s in range(2):
                        sl = 1 + 6 * c + 3 * s
                        nc.sync.dma_start(
                            out=btiles[b][:, sl:sl + 3, :],
                            in_=bias_d[b, c, :, 3 * s:3 * s + 3, :])
            # gpsimd ring: identity planes, mn, bv broadcast
            bvb = const.tile([128, 128], bf16)
            bv_ap = bv_d[:, :]
            bv_bcast = bass.AP(
                tensor=bv_ap.tensor,
                offset=bv_ap.offset,
                ap=[[0, 128]] + list(bv_ap.ap[1:]),
            )
            nc.gpsimd.dma_start(out=mn_ts[0], in_=mn_r[0])
            nc.gpsimd.dma_start(out=mn_ts[1], in_=mn_r[1])
            nc.gpsimd.dma_start(out=bvb, in_=bv_bcast)

            # ---- DVE-side constants ----
            negC = const.tile([128, 1], f32)
            nc.vector.memset(negC, -CSHIFT)
            for b in range(BPC):
                nc.vector.memset(vaugs[b][:, :, 128:129], 1.0)

            # residual prep on gpsimd (idle early)
            for b in range(BPC):
                for i in range(NB):
                    nc.gpsimd.tensor_add(mb_ts[b][:, i], mn_ts[b][:, i], bvb)

            # ---- projection helpers ----
            def proj_kq(b):
                mT_b = mT_ts[b]
                for h in range(2):
                    ps_kh = pqk.tile([128, 512], f32, name=f"ps_k{b}_{h}",
                                     tag="pqk")
                    nc.tensor.matmul(ps_kh, lhsT=wk_b, rhs=mT_b[:, ts(h, 512)],
                                     start=True, stop=True)
                    nc.vector.tensor_scalar_add(
                        kids[b][:, 0, ts(h, 512)], ps_kh,
                        mT_ts[b][:, N:N + 2].bitcast(f32))
                for h in range(2):
                    ps_qh = pqk.tile([128, 512], f32, name=f"ps_q{b}_{h}",
                                     tag="pqk")
                    nc.tensor.matmul(ps_qh, lhsT=wq_b, rhs=mT_b[:, ts(h, 512)],
                                     start=True, stop=True)
                    nc.vector.tensor_scalar_add(
                        btiles[b][:, 0, ts(h, 512)], ps_qh,
                        mT_ts[b][:, N + 2:N + 4].bitcast(f32))

            def proj_v(b):
                mT_b = mT_ts[b]
                for h in range(2):
                    ps_vh = pqk.tile([128, 512], f32, name=f"ps_v{b}_{h}",
                                     tag="pqk")
                    for i in range(4):
                        nc.tensor.matmul(ps_vh[:, ts(i, 128)],
                                         lhsT=mT_b[:, ts(4 * h + i, 128)],
                                         rhs=wv_b,
                                         start=(i == 0), stop=(i == 3),
                                         skip_group_check=True)
                    for i in range(4):
                        nc.vector.tensor_copy(vaugs[b][:, 4 * h + i, 0:128],
                                              ps_vh[:, ts(i, 128)])

            # ---- per-batch chunk machinery ----
            state = {}

            def setup_batch(b):
                bt = btiles[b]
                kid = kids[b]
                ps_os = [
                    pop.tile([128, 3, 132], f32, name=f"ps_o{b}_{p}", tag="po")
                    for p in range(3)
                ]
                base = bt[:, 0, 0]
                pstride = list(base.ap[0])
                ibase = kid[:, 1, 0]
                ii_w = bass.AP(tensor=ibase.tensor, offset=ibase.offset,
                               ap=[list(ibase.ap[0]), [128, 2], [1, 128]])
                state[b] = (bt, kid, ps_os, base, pstride, ii_w, [])

            def scores(b, c):
                bt, kid, ps_os, base, pstride, ii_w, Es = state[b]
                E = epool.tile([128, 2, N], fp8, name=f"E{b}_{c}", tag="E")
                for s in range(2):
                    j = 2 * c + s
                    slot = 1 + 6 * c + 3 * s
                    pss = [pqk.tile([128, 512], f32,
                                    name=f"ps_s{b}_{j}_{h}", tag="pqk")
                           for h in range(2)]
                    for h in range(2):
                        rhs1 = bass.AP(
                            tensor=base.tensor,
                            offset=base.offset + 512 * h,
                            ap=[pstride, [slot * N, 2], [1, 512]])
                        nc.tensor.matmul(pss[h],
                                         lhsT=kid[:, :, ts(j, 128)],
                                         rhs=rhs1, start=True, stop=False,
                                         perf_mode=DR,
                                         skip_group_check=True)
                    for h in range(2):
                        rhs2 = bass.AP(
                            tensor=base.tensor,
                            offset=base.offset + (slot + 1) * N + 512 * h,
                            ap=[pstride, [N, 2], [1, 512]])
                        nc.tensor.matmul(pss[h], lhsT=ii_w, rhs=rhs2,
                                         start=False, stop=True,
                                         perf_mode=DR,
                                         skip_group_check=True)
                    for h in range(2):
                        eh = E[:, s, ts(h, 512)]
                        if h == 0:
                            nc.scalar.activation(out=eh, in_=pss[h],
                                                 func=Exp, bias=negC,
                                                 scale=SCALE)
                        else:
                            nc.vector.tensor_scalar(
                                out=eh.bitcast(u8), in0=pss[h],
                                scalar1=EXP_A, scalar2=EXP_B,
                                op0=Alu.mult, op1=Alu.add)
                Es.append(E)

            def pv(b, c):
                bt, kid, ps_os, base, pstride, ii_w, Es = state[b]
                E = Es[c]
                va = vaugs[b]
                for i in range(NB):
                    nc.tensor.matmul(
                        ps_os[i // 3][:, i % 3, 0:129],
                        lhsT=E[:, :, ts(i, 128)],
                        rhs=va[:, 2 * c:2 * c + 2, 0:129],
                        start=(c == 0 and i % 3 == 0),
                        stop=(c == NCH - 1),
                        perf_mode=DR, skip_group_check=True)

            def tail(b):
                bt, kid, ps_os, base, pstride, ii_w, Es = state[b]
                ob = sb.tile([128, NB, H], bf16, name=f"ob{b}", tag="ob")
                mb_t = mb_ts[b]
                # all reciprocals first so the mul/add pipeline never
                # ping-pongs; muls split ACT/DVE, adds split gpsimd/DVE
                rs, o1s = [], []
                for i in range(NB):
                    ps_o = ps_os[i // 3][:, i % 3]
                    r = work.tile([128, 1], f32, name=f"r{b}_{i}", tag="r")
                    nc.vector.reciprocal(r, ps_o[:, 128:129])
                    rs.append(r)
                for i in range(NB):
                    ps_o = ps_os[i // 3][:, i % 3]
                    o1 = work.tile([128, 128], f32, name=f"o1_{b}_{i}",
                                   tag="o1")
                    nc.scalar.mul(o1, ps_o[:, 0:128], rs[i])
                    nc.vector.tensor_add(ob[:, i], o1, mb_t[:, i])
                    if i % 2 == 1:
                        nc.gpsimd.dma_start(out=out_r[b, :, i - 1:i + 1],
                                            in_=ob[:, i - 1:i + 1])

            # ---- schedule (v5 order): all projections up front; b1's
            # PV accumulators reuse b0's PSUM banks, so b1 is set up only
            # after b0's tail has read them
            proj_kq(0)
            setup_batch(0)
            scores(0, 0)
            proj_v(0)
            proj_kq(1)
            proj_v(1)
            for c in range(1, NCH):
                scores(0, c)
                pv(0, c - 1)
            pv(0, NCH - 1)
            tail(0)
            setup_batch(1)
            for c in range(NCH):
                scores(1, c)
                if c > 0:
                    pv(1, c - 1)
            pv(1, NCH - 1)
            tail(1)

    nc.compile()
    _CACHE["nc"] = nc
    return nc


def _shard_inputs(m, adj, dist, coulomb, Wq, bq, Wk, bk, Wv, bv):
    import ml_dtypes

    e4 = ml_dtypes.float8_e4m3
    bf = ml_dtypes.bfloat16

    wqT = np.ascontiguousarray(Wq.T).astype(bf)
    wkT = np.ascontiguousarray(Wk.T).astype(bf)
    wvT = np.ascontiguousarray(Wv.T).astype(bf)
    bv_s = bv.astype(bf).reshape(1, H)

    mT = np.ascontiguousarray(np.swapaxes(m, 1, 2)).astype(bf)
    bits = np.stack([bk, bq], axis=1).astype(np.float32).view(bf)  # [H, 4]
    ext = np.broadcast_to(bits[None], (B, H, 4))
    mT = np.ascontiguousarray(np.concatenate([mT, ext], axis=2))
    mn_b = np.ascontiguousarray(m).astype(bf)

    # bias: transpose to key-major, scale by sqrt(H), fp8, interleave to
    # [B, NCH, 128, 6(s*3+t), N] partition-major
    stack = np.stack([np.swapaxes(t, 1, 2) for t in (adj, dist, coulomb)],
                     axis=2)                       # [B, Nk, 3, Nq]
    stack = stack.reshape(B, NCH, 2, 128, 3, N)    # [B, c, s, p, t, n]
    stack = stack.transpose(0, 1, 3, 2, 4, 5)      # [B, c, p, s, t, n]
    biasq = np.ascontiguousarray(stack * INV).astype(e4)
    biasq = biasq.reshape(B, NCH, 128, 6, N)

    I8 = np.eye(128, dtype=np.float32).astype(e4)
    ii8 = np.ascontiguousarray(np.tile(I8, (1, NB)))

    in_maps = []
    for c in range(NCORES):
        sl = slice(c * BPC, (c + 1) * BPC)
        in_maps.append({
            "mT": mT[sl],
            "mn": mn_b[sl],
            "biasq": biasq[sl],
            "wq": wqT, "wk": wkT, "wv": wvT, "bv": bv_s,
            "ii8": ii8,
        })
    return in_maps


def run(trace=False, **inputs):
    _install_ntff_hook()
    from concourse.bass_utils import run_bass_kernel_spmd

    nc = _build()
    in_maps = _shard_inputs(**inputs)
    try:
        res = run_bass_kernel_spmd(nc, in_maps, core_ids=list(range(NCORES)),
                                   trace=trace)
    except Exception:
        # transient device errors (NRT_EXEC_UNIT_UNRECOVERABLE) have been
        # observed on this fabric; one retry usually succeeds
        res = run_bass_kernel_spmd(nc, in_maps, core_ids=list(range(NCORES)),
                                   trace=trace)
    out = np.concatenate([r["out"] for r in res.results], axis=0)
    return out.astype(np.float32), res


def kernel(**inputs):
    inputs = {k: np.asarray(v) for k, v in inputs.items()}
    out, _ = run(trace=False, **inputs)
    return out


# revision 23
# speedup vs baseline: 1.1585x; 1.1041x over previous
"""AtomAttention Trainium2 kernel (fp8 DoubleRow, v4).

reference:
    bias = adj + dist + coulomb                      # [B, N, N]
    q = m @ Wq.T + bq; k = m @ Wk.T + bk; v = m @ Wv.T + bv
    attn = softmax(q @ k.T / sqrt(H) + bias, axis=-1)
    out  = attn @ v + m                              # [B, N, H]

B=16, N=1024, H=128.  Data-parallel over batch: 2 batches per core on 8
NeuronCores.  Bias tensors dominate HBM traffic -> shipped pre-scaled by
sqrt(H) in fp8e4m3 (6 MB/core vs 24 MB f32); the 1/sqrt(H) is re-applied
inside the exp; a global shift C keeps E in fp8 range and cancels in
softmax.

Engine plan:
  - fp8 DoubleRow matmuls (two contractions per pass):
      DR1: (kT_j | I) x (qT | b0_j)   -> k.T q + b0
      DR2: (I | I)    x (b1_j | b2_j) -> + b1 + b2
    PV DoubleRows pair consecutive j-blocks; softmax denominator comes
    from a ones column in v.
  - scores live in single-bank [128,512] PSUM tiles (bufs=5) so several
    j-halves are in flight; exp of each half alternates between ACT
    (activation Exp) and DVE (Schraudolph bit trick: clamp(round(A*psum
    + B)) written as uint8 == fp8e4m3 bits of exp).  Total rel err
    ~5.4e-3 (gate 2e-2).
  - PV for chunk c issues after scores of chunk c+1 so the in-order PE
    never waits on exp; PV accumulates across chunks in 3 PSUM banks.
  - mega 25-slot bias tile per batch (slot 0 = qT) -> chunk DMAs never
    have WAR hazards; descriptor generation is spread across the three
    DMA rings (sync=bias, scalar=weights+mT, gpsimd=mn/identity/out).
"""

import sys
import types

import numpy as np

B, N, H = 16, 1024, 128
NB = N // 128   # 8 row blocks
NCH = 4         # chunks per batch, 2 blocks each
BPC = 2         # batches per core
NCORES = 8
SCALE = float(1.0 / np.sqrt(np.float32(H)))   # 0.08838835
INV = float(np.sqrt(np.float32(H)))           # 11.313708
CSHIFT = 3.25                                 # global exp shift
LOG2E = 1.4426950408889634
EXP_A = SCALE * LOG2E * 8.0                   # 1.0201033
EXP_B = 8.0 * (7.0 - CSHIFT * LOG2E) - 0.4    # Schraudolph, RNE convert

_CACHE = {}


def _install_ntff_hook():
    """The agent image's antenv lacks axon_hooks; register the NTFF
    profiling hook manually so trace=True yields exec_time_ns."""
    if "antenv.axon_hooks" in sys.modules:
        return
    try:
        import trn_agent_boot.trn_boot as tb

        hook = tb._ntff_profile_via_ctypes("/opt/axon/libaxon_pjrt.so")
    except Exception:
        hook = None
    mod = types.ModuleType("antenv.axon_hooks")
    mod.get_axon_ntff_profile_hook = lambda: hook
    mod.set_axon_ntff_profile_hook = lambda h: None
    sys.modules["antenv.axon_hooks"] = mod


def _build():
    if "nc" in _CACHE:
        return _CACHE["nc"]
    import concourse.bass as bass
    from concourse import bacc, mybir, tile

    f32 = mybir.dt.float32
    bf16 = mybir.dt.bfloat16
    fp8 = mybir.dt.float8e4
    u8 = mybir.dt.uint8
    ts = bass.ts
    DR = mybir.MatmulPerfMode.DoubleRow
    Exp = mybir.ActivationFunctionType.Exp
    Alu = mybir.AluOpType

    nc = bacc.Bacc("TRN2", target_bir_lowering=False, debug=False)

    # mT with f32 bk|bq appended as four bf16 columns (rides the same DMA)
    mT_d = nc.dram_tensor("mT", [BPC, 128, N + 4], bf16, kind="ExternalInput")
    mn_d = nc.dram_tensor("mn", [BPC, N, H], bf16, kind="ExternalInput")
    # [b, c, p, 6 slots (s*3+t), n] fp8, partition-major for 6KB/row DMA
    bias_d = nc.dram_tensor("biasq", [BPC, NCH, 128, 6, N], fp8,
                            kind="ExternalInput")
    ii8_d = nc.dram_tensor("ii8", [128, N], fp8, kind="ExternalInput")
    wq_d = nc.dram_tensor("wq", [H, H], bf16, kind="ExternalInput")
    wk_d = nc.dram_tensor("wk", [H, H], bf16, kind="ExternalInput")
    wv_d = nc.dram_tensor("wv", [H, H], bf16, kind="ExternalInput")
    bv_d = nc.dram_tensor("bv", [1, H], bf16, kind="ExternalInput")
    out_d = nc.dram_tensor("out", [BPC, N, H], bf16, kind="ExternalOutput")

    mn_r = mn_d.rearrange("b (i p) h -> b p i h", p=128)
    out_r = out_d.rearrange("b (i p) h -> b p i h", p=128)

    with tile.TileContext(nc) as tc:
        with (
            tc.tile_pool(name="const", bufs=1) as const,
            tc.tile_pool(name="cb", bufs=1) as cbp,
            tc.tile_pool(name="sb", bufs=2) as sb,
            tc.tile_pool(name="epool", bufs=3) as epool,
            tc.tile_pool(name="work", bufs=6) as work,
            tc.tile_pool(name="pqk", bufs=5, space="PSUM") as pqk,
            tc.tile_pool(name="po", bufs=3, space="PSUM") as pop,
        ):
            # ---- allocate big per-batch tiles ----
            btiles = [cbp.tile([128, 25, N], fp8, name=f"bt{b}")
                      for b in range(BPC)]
            kids = [cbp.tile([128, 2, N], fp8, name=f"kid{b}")
                    for b in range(BPC)]
            vaugs = [cbp.tile([128, NB, 132], fp8, name=f"va{b}")
                     for b in range(BPC)]
            mT_ts = [sb.tile([128, N + 4], bf16, name=f"mT{b}", tag="mT")
                     for b in range(BPC)]
            mn_ts = [sb.tile([128, NB, H], bf16, name=f"mn{b}", tag="mn")
                     for b in range(BPC)]
            mb_ts = [sb.tile([128, NB, H], bf16, name=f"mb{b}", tag="mb")
                     for b in range(BPC)]

            # ---- descriptor generation spread across the 3 DMA rings ----
            # scalar ring: bf16 weights (tiny, land first)
            wk_b = const.tile([128, 128], bf16)
            wq_b = const.tile([128, 128], bf16)
            wv_b = const.tile([128, 128], bf16)
            nc.scalar.dma_start(out=wk_b, in_=wk_d[:, :])
            nc.scalar.dma_start(out=wq_b, in_=wq_d[:, :])
            nc.scalar.dma_start(out=wv_b, in_=wv_d[:, :])
            # sync ring: identity planes + mT ahead of the bias flood
            # (queue FIFO), then bias in half-chunks for deep pipelining
            nc.sync.dma_start(out=kids[0][:, 1, :], in_=ii8_d[:, :])
            nc.sync.dma_start(out=mT_ts[0], in_=mT_d[0])
            nc.sync.dma_start(out=kids[1][:, 1, :], in_=ii8_d[:, :])
            nc.sync.dma_start(out=mT_ts[1], in_=mT_d[1])
            for b in range(BPC):
                for c in range(NCH):
                    for s in range(2):
                        sl = 1 + 6 * c + 3 * s
                        nc.sync.dma_start(
                            out=btiles[b][:, sl:sl + 3, :],
                            in_=bias_d[b, c, :, 3 * s:3 * s + 3, :])
            # gpsimd ring: identity planes, mn, bv broadcast
            bvb = const.tile([128, 128], bf16)
            bv_ap = bv_d[:, :]
            bv_bcast = bass.AP(
                tensor=bv_ap.tensor,
                offset=bv_ap.offset,
                ap=[[0, 128]] + list(bv_ap.ap[1:]),
            )
            nc.gpsimd.dma_start(out=mn_ts[0], in_=mn_r[0])
            nc.gpsimd.dma_start(out=mn_ts[1], in_=mn_r[1])
            nc.gpsimd.dma_start(out=bvb, in_=bv_bcast)

            # ---- DVE-side constants ----
            negC = const.tile([128, 1], f32)
            nc.vector.memset(negC, -CSHIFT)
            for b in range(BPC):
                nc.vector.memset(vaugs[b][:, :, 128:129], 1.0)

            # residual prep on gpsimd (idle early)
            for b in range(BPC):
                for i in range(NB):
                    nc.gpsimd.tensor_add(mb_ts[b][:, i], mn_ts[b][:, i], bvb)

            # ---- projection helpers ----
            def proj_kq(b):
                mT_b = mT_ts[b]
                for h in range(2):
                    ps_kh = pqk.tile([128, 512], f32, name=f"ps_k{b}_{h}",
                                     tag="pqk")
                    nc.tensor.matmul(ps_kh, lhsT=wk_b, rhs=mT_b[:, ts(h, 512)],
                                     start=True, stop=True)
                    nc.scalar.add(
                        kids[b][:, 0, ts(h, 512)], ps_kh,
                        mT_ts[b][:, N:N + 2].bitcast(f32))
                for h in range(2):
                    ps_qh = pqk.tile([128, 512], f32, name=f"ps_q{b}_{h}",
                                     tag="pqk")
                    nc.tensor.matmul(ps_qh, lhsT=wq_b, rhs=mT_b[:, ts(h, 512)],
                                     start=True, stop=True)
                    nc.vector.tensor_scalar_add(
                        btiles[b][:, 0, ts(h, 512)], ps_qh,
                        mT_ts[b][:, N + 2:N + 4].bitcast(f32))

            def proj_v(b):
                mT_b = mT_ts[b]
                for h in range(2):
                    ps_vh = pqk.tile([128, 512], f32, name=f"ps_v{b}_{h}",
                                     tag="pqk")
                    for i in range(4):
                        nc.tensor.matmul(ps_vh[:, ts(i, 128)],
                                         lhsT=mT_b[:, ts(4 * h + i, 128)],
                                         rhs=wv_b,
                                         start=(i == 0), stop=(i == 3),
                                         skip_group_check=True)
                    for i in range(4):
                        nc.vector.tensor_copy(vaugs[b][:, 4 * h + i, 0:128],
                                              ps_vh[:, ts(i, 128)])

            # ---- per-batch chunk machinery ----
            state = {}

            def setup_batch(b):
                bt = btiles[b]
                kid = kids[b]
                ps_os = [
                    pop.tile([128, 3, 132], f32, name=f"ps_o{b}_{p}", tag="po")
                    for p in range(3)
                ]
                base = bt[:, 0, 0]
                pstride = list(base.ap[0])
                ibase = kid[:, 1, 0]
                ii_w = bass.AP(tensor=ibase.tensor, offset=ibase.offset,
                               ap=[list(ibase.ap[0]), [128, 2], [1, 128]])
                state[b] = (bt, kid, ps_os, base, pstride, ii_w, [])

            def scores(b, c):
                bt, kid, ps_os, base, pstride, ii_w, Es = state[b]
                E = epool.tile([128, 2, N], fp8, name=f"E{b}_{c}", tag="E")
                for s in range(2):
                    j = 2 * c + s
                    slot = 1 + 6 * c + 3 * s
                    pss = [pqk.tile([128, 512], f32,
                                    name=f"ps_s{b}_{j}_{h}", tag="pqk")
                           for h in range(2)]
                    for h in range(2):
                        rhs1 = bass.AP(
                            tensor=base.tensor,
                            offset=base.offset + 512 * h,
                            ap=[pstride, [slot * N, 2], [1, 512]])
                        nc.tensor.matmul(pss[h],
                                         lhsT=kid[:, :, ts(j, 128)],
                                         rhs=rhs1, start=True, stop=False,
                                         perf_mode=DR,
                                         skip_group_check=True)
                    for h in range(2):
                        rhs2 = bass.AP(
                            tensor=base.tensor,
                            offset=base.offset + (slot + 1) * N + 512 * h,
                            ap=[pstride, [N, 2], [1, 512]])
                        nc.tensor.matmul(pss[h], lhsT=ii_w, rhs=rhs2,
                                         start=False, stop=True,
                                         perf_mode=DR,
                                         skip_group_check=True)
                    for h in range(2):
                        eh = E[:, s, ts(h, 512)]
                        if h == 0:
                            nc.scalar.activation(out=eh, in_=pss[h],
                                                 func=Exp, bias=negC,
                                                 scale=SCALE)
                        else:
                            nc.vector.tensor_scalar(
                                out=eh.bitcast(u8), in0=pss[h],
                                scalar1=EXP_A, scalar2=EXP_B,
                                op0=Alu.mult, op1=Alu.add)
                Es.append(E)

            def pv(b, c):
                bt, kid, ps_os, base, pstride, ii_w, Es = state[b]
                E = Es[c]
                va = vaugs[b]
                for i in range(NB):
                    nc.tensor.matmul(
                        ps_os[i // 3][:, i % 3, 0:129],
                        lhsT=E[:, :, ts(i, 128)],
                        rhs=va[:, 2 * c:2 * c + 2, 0:129],
                        start=(c == 0 and i % 3 == 0),
                        stop=(c == NCH - 1),
                        perf_mode=DR, skip_group_check=True)

            def tail(b):
                bt, kid, ps_os, base, pstride, ii_w, Es = state[b]
                ob = sb.tile([128, NB, H], bf16, name=f"ob{b}", tag="ob")
                mb_t = mb_ts[b]
                # all reciprocals first so the mul/add pipeline never
                # ping-pongs; muls split ACT/DVE, adds split gpsimd/DVE
                rs, o1s = [], []
                for i in range(NB):
                    ps_o = ps_os[i // 3][:, i % 3]
                    r = work.tile([128, 1], f32, name=f"r{b}_{i}", tag="r")
                    nc.vector.reciprocal(r, ps_o[:, 128:129])
                    rs.append(r)
                for i in range(NB):
                    ps_o = ps_os[i // 3][:, i % 3]
                    o1 = work.tile([128, 128], f32, name=f"o1_{b}_{i}",
                                   tag="o1")
                    nc.scalar.mul(o1, ps_o[:, 0:128], rs[i])
                    nc.vector.tensor_add(ob[:, i], o1, mb_t[:, i])
                    if i % 2 == 1:
                        nc.gpsimd.dma_start(out=out_r[b, :, i - 1:i + 1],
                                            in_=ob[:, i - 1:i + 1])

            # ---- schedule (v5 order): all projections up front; b1's
            # PV accumulators reuse b0's PSUM banks, so b1 is set up only
            # after b0's tail has read them
            proj_kq(0)
            proj_v(0)
            proj_kq(1)
            proj_v(1)
            setup_batch(0)
            scores(0, 0)
            for c in range(1, NCH):
                scores(0, c)
                pv(0, c - 1)
            pv(0, NCH - 1)
            tail(0)
            setup_batch(1)
            for c in range(NCH):
                scores(1, c)
                if c > 0:
                    pv(1, c - 1)
            pv(1, NCH - 1)
            tail(1)

    nc.compile()
    _CACHE["nc"] = nc
    return nc


def _shard_inputs(m, adj, dist, coulomb, Wq, bq, Wk, bk, Wv, bv):
    import ml_dtypes

    e4 = ml_dtypes.float8_e4m3
    bf = ml_dtypes.bfloat16

    wqT = np.ascontiguousarray(Wq.T).astype(bf)
    wkT = np.ascontiguousarray(Wk.T).astype(bf)
    wvT = np.ascontiguousarray(Wv.T).astype(bf)
    bv_s = bv.astype(bf).reshape(1, H)

    mT = np.ascontiguousarray(np.swapaxes(m, 1, 2)).astype(bf)
    bits = np.stack([bk, bq], axis=1).astype(np.float32).view(bf)  # [H, 4]
    ext = np.broadcast_to(bits[None], (B, H, 4))
    mT = np.ascontiguousarray(np.concatenate([mT, ext], axis=2))
    mn_b = np.ascontiguousarray(m).astype(bf)

    # bias: transpose to key-major, scale by sqrt(H), fp8, interleave to
    # [B, NCH, 128, 6(s*3+t), N] partition-major
    stack = np.stack([np.swapaxes(t, 1, 2) for t in (adj, dist, coulomb)],
                     axis=2)                       # [B, Nk, 3, Nq]
    stack = stack.reshape(B, NCH, 2, 128, 3, N)    # [B, c, s, p, t, n]
    stack = stack.transpose(0, 1, 3, 2, 4, 5)      # [B, c, p, s, t, n]
    biasq = np.ascontiguousarray(stack * INV).astype(e4)
    biasq = biasq.reshape(B, NCH, 128, 6, N)

    I8 = np.eye(128, dtype=np.float32).astype(e4)
    ii8 = np.ascontiguousarray(np.tile(I8, (1, NB)))

    in_maps = []
    for c in range(NCORES):
        sl = slice(c * BPC, (c + 1) * BPC)
        in_maps.append({
            "mT": mT[sl],
            "mn": mn_b[sl],
            "biasq": biasq[sl],
            "wq": wqT, "wk": wkT, "wv": wvT, "bv": bv_s,
            "ii8": ii8,
        })
    return in_maps


def run(trace=False, **inputs):
    _install_ntff_hook()
    from concourse.bass_utils import run_bass_kernel_spmd

    nc = _build()
    in_maps = _shard_inputs(**inputs)
    try:
        res = run_bass_kernel_spmd(nc, in_maps, core_ids=list(range(NCORES)),
                                   trace=trace)
    except Exception:
        # transient device errors (NRT_EXEC_UNIT_UNRECOVERABLE) have been
        # observed on this fabric; one retry usually succeeds
        res = run_bass_kernel_spmd(nc, in_maps, core_ids=list(range(NCORES)),
                                   trace=trace)
    out = np.concatenate([r["out"] for r in res.results], axis=0)
    return out.astype(np.float32), res


def kernel(**inputs):
    inputs = {k: np.asarray(v) for k, v in inputs.items()}
    out, _ = run(trace=False, **inputs)
    return out
